# revision 1
# baseline (speedup 1.0000x reference)
"""RNN-T JointNetwork kernel for Trainium2 (Bass/Tile), SPMD over 8 NeuronCores.

Computes, per batch element b (one per core):
    h_enc = x_enc[b] @ w_l + b_l          # (T, H)
    h_prd = x_prd[b] @ w_p + b_p          # (U, H)
    h     = tanh(h_enc[t] + h_prd[u])     # (T, U, H)
    out   = h @ w_h + b_h                 # (T, U, V)

Layout strategy (per core):
  * Everything upstream of the big GEMM is kept feature-major ("h on
    partitions"): h_encT [H, T], h_prdT [H, U], so that h tiles are directly
    usable as the stationary (lhsT) operand of the output GEMM.
  * Rows of the big GEMM are ordered u-major: r' = u*T + t.  For a fixed u,
    h.T[:, u, :] = tanh(h_encT + h_prdT[:, u]) is ONE scalar-engine
    activation op (bias = per-partition column h_prdT[:, u]), fusing the
    broadcast-add and tanh and keeping the vector engine free for the
    PSUM+bias epilogue of the big GEMM.
  * Big GEMM uses float32r (full-rate fp32 matmul at free-dim>=256) with V
    split into two 512-wide PSUM banks, accumulating over 4 k-tiles of H.
  * Output rows r' = u*T + t map to logits rows r = t*U + u; each 128-row
    output tile is stored with <=2 DMAs (one per u-segment), each writing
    contiguous 4KB rows at a fixed stride.
  * Emission order matters for pipeline fill: x/w_l loads precede w_p/w_h
    loads; the first u-chunk is small (CU=2) so PE starts the big GEMM
    early; per-u activation ops are emitted du-outer so the first GEMM
    tile's inputs are ready after 4 ACT ops, not 3/4 of the chunk.
"""

import sys

for _p in ("/opt/trn_rl_repo",):
    if _p not in sys.path:
        sys.path.insert(0, _p)

import numpy as np

B, T, U = 8, 200, 50
E = H = 512
V = 1024
P = 128
KT = E // P  # 4 contraction tiles for the small GEMMs
HT = H // P  # 4 contraction tiles for the big GEMM
R = T * U    # rows per core
N_CORES = 8
CHUNKS = [2, 16, 16, 16]  # u-chunks; first small to fill the pipeline fast

_CACHE = {}
_last_in_maps = None


def _emit(nc, tc, tile, mybir):
    f32 = mybir.dt.float32
    f32r = mybir.dt.float32r
    Act = mybir.ActivationFunctionType

    x_enc_d = nc.dram_tensor("x_enc", [T, E], f32, kind="ExternalInput")
    x_prd_d = nc.dram_tensor("x_prd", [U, E], f32, kind="ExternalInput")
    w_l_d = nc.dram_tensor("w_l", [E, H], f32, kind="ExternalInput")
    b_l_d = nc.dram_tensor("b_l", [H], f32, kind="ExternalInput")
    w_p_d = nc.dram_tensor("w_p", [E, H], f32, kind="ExternalInput")
    b_p_d = nc.dram_tensor("b_p", [H], f32, kind="ExternalInput")
    w_h_d = nc.dram_tensor("w_h", [H, V], f32, kind="ExternalInput")
    b_h_d = nc.dram_tensor("b_h", [V], f32, kind="ExternalInput")
    out_d = nc.dram_tensor("out", [R, V], f32, kind="ExternalOutput")

    from concourse.masks import make_identity
    from contextlib import ExitStack

    ctx = ExitStack()
    cpool = ctx.enter_context(tc.tile_pool(name="const", bufs=1))
    pbig = ctx.enter_context(tc.tile_pool(name="pbig", bufs=4, space="PSUM"))
    hcpool = ctx.enter_context(tc.tile_pool(name="hc", bufs=2))
    opool = ctx.enter_context(tc.tile_pool(name="op", bufs=6))

    ident = cpool.tile([P, P], f32, tag="ident")
    make_identity(nc, ident[:])

    # ---- inputs that gate the PE pipeline come first ----
    xe_nat = []
    t_sizes = []
    t0 = 0
    while t0 < T:
        ti = min(P, T - t0)
        t_ = cpool.tile([P, E], f32, tag=f"xen{len(xe_nat)}",
                        name=f"xen{len(xe_nat)}")
        nc.sync.dma_start(out=t_[:ti, :], in_=x_enc_d[t0:t0 + ti, :])
        xe_nat.append(t_)
        t_sizes.append(ti)
        t0 += ti
    xp_nat = cpool.tile([P, E], f32, tag="xpn")
    nc.sync.dma_start(out=xp_nat[:U, :], in_=x_prd_d[:, :])

    wl = []
    for k in range(KT):
        t_ = cpool.tile([P, H], f32, tag=f"wl{k}", name=f"wl{k}")
        nc.sync.dma_start(out=t_[:], in_=w_l_d[k * P:(k + 1) * P, :])
        wl.append(t_)
    bl = cpool.tile([P, KT], f32, tag="bl")
    nc.sync.dma_start(out=bl[:], in_=b_l_d[:].rearrange("(a p) -> p a", p=P))
    wp = []
    for k in range(KT):
        t_ = cpool.tile([P, H], f32, tag=f"wp{k}", name=f"wp{k}")
        nc.sync.dma_start(out=t_[:], in_=w_p_d[k * P:(k + 1) * P, :])
        wp.append(t_)
    bp = cpool.tile([P, KT], f32, tag="bp")
    nc.sync.dma_start(out=bp[:], in_=b_p_d[:].rearrange("(a p) -> p a", p=P))

    # ---- transpose x_enc / x_prd on the PE (feature dim -> partitions) ----
    xeT = [cpool.tile([P, T], f32, tag=f"xeT{k}", name=f"xeT{k}")
           for k in range(KT)]
    xpT = [cpool.tile([P, U], f32, tag=f"xpT{k}", name=f"xpT{k}")
           for k in range(KT)]
    _rr = [0]
    def _pstile(shape):
        _rr[0] ^= 1
        return pbig.tile(shape, f32, tag=f"ps{_rr[0]}", name="pss")

    for k in range(KT):
        t0 = 0
        for i, ti in enumerate(t_sizes):
            ps = _pstile([P, 512])
            nc.tensor.transpose(
                ps[:, :ti], xe_nat[i][:ti, k * P:(k + 1) * P], ident[:ti, :ti]
            )
            nc.scalar.copy(xeT[k][:, t0:t0 + ti], ps[:, :ti])
            t0 += ti
        ps = _pstile([P, 512])
        nc.tensor.transpose(
            ps[:, :U], xp_nat[:U, k * P:(k + 1) * P], ident[:U, :U]
        )
        nc.scalar.copy(xpT[k][:, :U], ps[:, :U])

    # ---- small GEMMs: h_encT [H, T], h_prdT [H, U] (+bias via ACT) ----
    heT = [cpool.tile([P, T], f32, tag=f"heT{j}", name=f"heT{j}")
           for j in range(HT)]
    hpT = [cpool.tile([P, U], f32, tag=f"hpT{j}", name=f"hpT{j}")
           for j in range(HT)]
    for j in range(HT):
        ps = _pstile([P, 512])
        for k in range(KT):
            nc.tensor.matmul(
                ps[:, :T],
                wl[k][:, j * P:(j + 1) * P],
                xeT[k][:, :T],
                start=(k == 0),
                stop=(k == KT - 1),
            )
        nc.scalar.activation(
            heT[j][:], ps[:, :T], Act.Identity, bias=bl[:, j:j + 1]
        )
    for j in range(HT):
        ps = _pstile([P, 512])
        for k in range(KT):
            nc.tensor.matmul(
                ps[:, :U],
                wp[k][:, j * P:(j + 1) * P],
                xpT[k][:, :U],
                start=(k == 0),
                stop=(k == KT - 1),
            )
        nc.scalar.activation(
            hpT[j][:], ps[:, :U], Act.Identity, bias=bp[:, j:j + 1]
        )

    # ---- big-GEMM weights last: not needed until the first chunk's GEMM ----
    wh = []
    for k in range(HT):
        ts_ = cpool.tile([P, V], f32, tag="whs", bufs=2, name="whs")
        nc.sync.dma_start(out=ts_[:], in_=w_h_d[k * P:(k + 1) * P, :])
        t_ = cpool.tile([P, V], f32r, tag=f"wh{k}", name=f"wh{k}")
        nc.vector.tensor_copy(out=t_[:], in_=ts_[:])
        wh.append(t_)
    bh_rep = cpool.tile([P, V], f32, tag="bh")
    nc.sync.dma_start(
        out=bh_rep[:], in_=b_h_d[:].unsqueeze(0).broadcast_to([P, V])
    )

    # ---- main loop over u-chunks; rows r' = u*T + t ----
    out_view = out_d[:].rearrange("(t u) v -> u t v", u=U)
    max_cu = max(CHUNKS)
    u0 = 0
    for cu in CHUNKS:
        rc = cu * T
        hc = [hcpool.tile([P, max_cu * T], f32r, tag=f"hc{j}", name=f"hc{j}")
              for j in range(HT)]
        # fused broadcast-add + tanh; du-outer so early GEMM tiles unblock
        for du in range(cu):
            for j in range(HT):
                nc.scalar.activation(
                    hc[j][:, du * T:(du + 1) * T],
                    heT[j][:, :T],
                    Act.Tanh,
                    bias=hpT[j][:, u0 + du:u0 + du + 1],
                )
        # big GEMM over 128-row tiles of this chunk
        for m0 in range(0, rc, P):
            m = min(P, rc - m0)
            ps0 = pbig.tile([P, 512], f32, tag="ps0")
            ps1 = pbig.tile([P, 512], f32, tag="ps1")
            for j in range(HT):
                lhsT = hc[j][:, m0:m0 + m]
                nc.tensor.matmul(
                    ps0[:m, :], lhsT, wh[j][:, 0:512],
                    start=(j == 0), stop=(j == HT - 1),
                )
                nc.tensor.matmul(
                    ps1[:m, :], lhsT, wh[j][:, 512:V],
                    start=(j == 0), stop=(j == HT - 1),
                )
            # epilogue per V-half so each PSUM bank drains + stores
            # independently; store rows split at u boundaries (<=2 segs)
            for v, psv in ((0, ps0), (1, ps1)):
                ot = opool.tile([P, 512], f32, tag=f"ot{v}", name=f"ot{v}")
                nc.vector.tensor_add(
                    ot[:m, :], psv[:m, :], bh_rep[:m, v * 512:(v + 1) * 512]
                )
                seg = m0
                while seg < m0 + m:
                    du = seg // T
                    tA = seg % T
                    seg_len = min(m0 + m, (du + 1) * T) - seg
                    nc.sync.dma_start(
                        out=out_view[
                            u0 + du, tA:tA + seg_len, v * 512:(v + 1) * 512
                        ],
                        in_=ot[seg - m0:seg - m0 + seg_len, :],
                    )
                    seg += seg_len
        u0 += cu

    ctx.close()


def _build():
    if "nc" in _CACHE:
        return _CACHE["nc"]
    from concourse import bacc, mybir
    import concourse.tile as tile

    nc = bacc.Bacc("TRN2", target_bir_lowering=False, debug=False)
    with tile.TileContext(nc) as tc:
        _emit(nc, tc, tile, mybir)
    nc.compile()
    _CACHE["nc"] = nc
    return nc


def kernel(**inputs):
    from concourse.bass_utils import run_bass_kernel_spmd

    nc = _build()
    x_enc = np.ascontiguousarray(np.asarray(inputs["x_enc"], dtype=np.float32))
    x_prd = np.ascontiguousarray(np.asarray(inputs["x_prd"], dtype=np.float32))
    shared = {
        "w_l": np.ascontiguousarray(np.asarray(inputs["w_l"], np.float32)),
        "b_l": np.ascontiguousarray(np.asarray(inputs["b_l"], np.float32)),
        "w_p": np.ascontiguousarray(np.asarray(inputs["w_p"], np.float32)),
        "b_p": np.ascontiguousarray(np.asarray(inputs["b_p"], np.float32)),
        "w_h": np.ascontiguousarray(np.asarray(inputs["w_h"], np.float32)),
        "b_h": np.ascontiguousarray(np.asarray(inputs["b_h"], np.float32)),
    }
    in_maps = []
    for b in range(N_CORES):
        m = dict(shared)
        m["x_enc"] = np.ascontiguousarray(x_enc[b, :, 0, :])
        m["x_prd"] = np.ascontiguousarray(x_prd[b, 0, :, :])
        in_maps.append(m)

    global _last_in_maps
    _last_in_maps = in_maps
    res = run_bass_kernel_spmd(nc, in_maps, core_ids=list(range(N_CORES)))
    out = np.stack(
        [res.results[b]["out"].reshape(T, U, V) for b in range(N_CORES)], axis=0
    )
    return out



# revision 2
# speedup vs baseline: 1.4608x; 1.4608x over previous
"""RNN-T JointNetwork kernel for Trainium2 (Bass/Tile), SPMD over 8 NeuronCores.

Computes, per batch element b (one per core):
    h_enc = x_enc[b] @ w_l + b_l          # (T, H)
    h_prd = x_prd[b] @ w_p + b_p          # (U, H)
    h     = tanh(h_enc[t] + h_prd[u])     # (T, U, H)
    out   = h @ w_h + b_h                 # (T, U, V)

Layout strategy (per core):
  * x_enc/x_prd arrive HOST-TRANSPOSED and in bf16: xeT [E, T], xpT [E, U]
    DMA straight into feature-major SBUF slabs - no PE transposes, no
    identity, no PSUM/ACT copies on the critical fill path.
  * Small GEMMs run in bf16 (1 cycle/row on the PE at any free size, vs 4
    for fp32r under 256); h_encT/h_prdT epilogues keep f32 for accuracy.
  * Rows of the big GEMM are ordered u-major: r' = u*T + t.  For a fixed u,
    h.T[:, u, :] = tanh(h_encT + h_prdT[:, u]) is ONE scalar-engine
    activation op (bias = per-partition column h_prdT[:, u]), fusing the
    broadcast-add and tanh; output hc is bf16 (PE-ready lhsT).
  * Big GEMM: bf16 stationary (hc) x bf16 moving (w_h), f32 PSUM, V split
    into two 512-wide PSUM banks, 4 k-tiles of H, 4 output tiles in flight
    (tags ps0/ps1 x bufs=4 = all 8 PSUM banks).
  * Epilogue: DVE adds b_h and narrows to fp16 into one [128, V] tile;
    output DMA moves HALF the bytes of an f32 kernel.  DMA transfers are a
    serial resource at ~360 GB/s aggregate, and the f32 output stream was
    the previous bottleneck (out DMA busy > PE busy).
  * Output rows r' = u*T + t map to logits rows r = t*U + u; each 128-row
    tile stores with <=2 DMAs (one per u-segment), 2 KB contiguous rows.

Host runner (wall-clock):
  * Caches the jitted shard_map executable across calls (no retrace).
  * Donated output buffers are created ON DEVICE by a tiny separate jit
    (the bass_exec HLO module must stay params-only), so the host never
    uploads zero-filled output-sized buffers.
  * Inputs ship as bf16 (x, w) + f32 biases; output returns as fp16 and is
    upcast host-side.  Per call: ~18 MB up + ~164 MB down instead of
    ~364 MB up + ~327 MB down.
"""

import sys

for _p in ("/opt/trn_rl_repo",):
    if _p not in sys.path:
        sys.path.insert(0, _p)

import numpy as np
import ml_dtypes

B, T, U = 8, 200, 50
E = H = 512
V = 1024
P = 128
KT = E // P  # 4 contraction tiles for the small GEMMs
HT = H // P  # 4 contraction tiles for the big GEMM
R = T * U    # rows per core
N_CORES = 8
CHUNKS = [2, 16, 16, 16]  # u-chunks; first small to fill the pipeline fast

_CACHE = {}
_last_in_maps = None


def _emit(nc, tc, tile, mybir):
    f32 = mybir.dt.float32
    bf16 = mybir.dt.bfloat16
    f16 = mybir.dt.float16
    Act = mybir.ActivationFunctionType

    xeT_d = nc.dram_tensor("xeT", [E, T], bf16, kind="ExternalInput")
    xpT_d = nc.dram_tensor("xpT", [E, U], bf16, kind="ExternalInput")
    w_l_d = nc.dram_tensor("w_l", [E, H], bf16, kind="ExternalInput")
    b_l_d = nc.dram_tensor("b_l", [H], f32, kind="ExternalInput")
    w_p_d = nc.dram_tensor("w_p", [E, H], bf16, kind="ExternalInput")
    b_p_d = nc.dram_tensor("b_p", [H], f32, kind="ExternalInput")
    w_h_d = nc.dram_tensor("w_h", [H, V], bf16, kind="ExternalInput")
    b_h_d = nc.dram_tensor("b_h", [V], f32, kind="ExternalInput")
    out_d = nc.dram_tensor("out", [R, V], f16, kind="ExternalOutput")

    from contextlib import ExitStack

    ctx = ExitStack()
    cpool = ctx.enter_context(tc.tile_pool(name="const", bufs=1))
    pbig = ctx.enter_context(tc.tile_pool(name="pbig", bufs=4, space="PSUM"))
    hcpool = ctx.enter_context(tc.tile_pool(name="hc", bufs=2))
    opool = ctx.enter_context(tc.tile_pool(name="op", bufs=6))

    # ---- input DMAs, ordered by first use (DMA engines are serial) ----
    xeT = []
    for k in range(KT):
        t_ = cpool.tile([P, T], bf16, tag=f"xeT{k}", name=f"xeT{k}")
        nc.sync.dma_start(out=t_[:], in_=xeT_d[k * P:(k + 1) * P, :])
        xeT.append(t_)
    wl = []
    for k in range(KT):
        t_ = cpool.tile([P, H], bf16, tag=f"wl{k}", name=f"wl{k}")
        nc.sync.dma_start(out=t_[:], in_=w_l_d[k * P:(k + 1) * P, :])
        wl.append(t_)
    bl = cpool.tile([P, KT], f32, tag="bl")
    nc.sync.dma_start(out=bl[:], in_=b_l_d[:].rearrange("(a p) -> p a", p=P))
    xpT = []
    for k in range(KT):
        t_ = cpool.tile([P, U], bf16, tag=f"xpT{k}", name=f"xpT{k}")
        nc.sync.dma_start(out=t_[:], in_=xpT_d[k * P:(k + 1) * P, :])
        xpT.append(t_)
    wp = []
    for k in range(KT):
        t_ = cpool.tile([P, H], bf16, tag=f"wp{k}", name=f"wp{k}")
        nc.sync.dma_start(out=t_[:], in_=w_p_d[k * P:(k + 1) * P, :])
        wp.append(t_)
    bp = cpool.tile([P, KT], f32, tag="bp")
    nc.sync.dma_start(out=bp[:], in_=b_p_d[:].rearrange("(a p) -> p a", p=P))
    wh = []
    for k in range(HT):
        t_ = cpool.tile([P, V], bf16, tag=f"wh{k}", name=f"wh{k}")
        nc.sync.dma_start(out=t_[:], in_=w_h_d[k * P:(k + 1) * P, :])
        wh.append(t_)
    bh_rep = cpool.tile([P, V], f32, tag="bh")
    nc.sync.dma_start(
        out=bh_rep[:], in_=b_h_d[:].unsqueeze(0).broadcast_to([P, V])
    )

    # ---- small GEMMs: h_encT [H, T], h_prdT [H, U] (+bias via ACT) ----
    heT = [cpool.tile([P, T], f32, tag=f"heT{j}", name=f"heT{j}")
           for j in range(HT)]
    hpT = [cpool.tile([P, U], f32, tag=f"hpT{j}", name=f"hpT{j}")
           for j in range(HT)]
    for j in range(HT):
        ps = pbig.tile([P, 512], f32, tag=f"ps{j % 2}", name="pss")
        for k in range(KT):
            nc.tensor.matmul(
                ps[:, :T],
                wl[k][:, j * P:(j + 1) * P],
                xeT[k][:, :T],
                start=(k == 0),
                stop=(k == KT - 1),
            )
        nc.scalar.activation(
            heT[j][:], ps[:, :T], Act.Identity, bias=bl[:, j:j + 1]
        )
    for j in range(HT):
        ps = pbig.tile([P, 512], f32, tag=f"ps{j % 2}", name="pss")
        for k in range(KT):
            nc.tensor.matmul(
                ps[:, :U],
                wp[k][:, j * P:(j + 1) * P],
                xpT[k][:, :U],
                start=(k == 0),
                stop=(k == KT - 1),
            )
        nc.scalar.activation(
            hpT[j][:], ps[:, :U], Act.Identity, bias=bp[:, j:j + 1]
        )

    # ---- main loop over u-chunks; rows r' = u*T + t ----
    out_view = out_d[:].rearrange("(t u) v -> u t v", u=U)
    max_cu = max(CHUNKS)
    u0 = 0
    for cu in CHUNKS:
        rc = cu * T
        hc = [hcpool.tile([P, max_cu * T], bf16, tag=f"hc{j}", name=f"hc{j}")
              for j in range(HT)]
        # fused broadcast-add + tanh; du-outer so early GEMM tiles unblock
        for du in range(cu):
            for j in range(HT):
                nc.scalar.activation(
                    hc[j][:, du * T:(du + 1) * T],
                    heT[j][:, :T],
                    Act.Tanh,
                    bias=hpT[j][:, u0 + du:u0 + du + 1],
                )
        # big GEMM over 128-row tiles of this chunk
        for m0 in range(0, rc, P):
            m = min(P, rc - m0)
            ps0 = pbig.tile([P, 512], f32, tag="ps0")
            ps1 = pbig.tile([P, 512], f32, tag="ps1")
            for j in range(HT):
                lhsT = hc[j][:, m0:m0 + m]
                nc.tensor.matmul(
                    ps0[:m, :], lhsT, wh[j][:, 0:512],
                    start=(j == 0), stop=(j == HT - 1),
                )
                nc.tensor.matmul(
                    ps1[:m, :], lhsT, wh[j][:, 512:V],
                    start=(j == 0), stop=(j == HT - 1),
                )
            # epilogue: bias add + fp16 narrowing into one [P, V] tile,
            # then <=2 store DMAs (split at u boundaries)
            ot = opool.tile([P, V], f16, tag="ot", name="ot")
            nc.vector.tensor_add(
                ot[:m, 0:512], ps0[:m, :], bh_rep[:m, 0:512]
            )
            nc.vector.tensor_add(
                ot[:m, 512:V], ps1[:m, :], bh_rep[:m, 512:V]
            )
            seg = m0
            while seg < m0 + m:
                du = seg // T
                tA = seg % T
                seg_len = min(m0 + m, (du + 1) * T) - seg
                nc.sync.dma_start(
                    out=out_view[u0 + du, tA:tA + seg_len, :],
                    in_=ot[seg - m0:seg - m0 + seg_len, :],
                )
                seg += seg_len
        u0 += cu

    ctx.close()


def _build():
    if "nc" in _CACHE:
        return _CACHE["nc"]
    from concourse import bacc, mybir
    import concourse.tile as tile

    nc = bacc.Bacc("TRN2", target_bir_lowering=False, debug=False)
    with tile.TileContext(nc) as tc:
        _emit(nc, tc, tile, mybir)
    nc.compile()
    _CACHE["nc"] = nc
    return nc


def _get_exec():
    """Build (once) the cached jitted shard_map executable + device-zeros fn."""
    if "exec" in _CACHE:
        return _CACHE["exec"]
    import jax
    import jax.numpy as jnp
    from jax.experimental.shard_map import shard_map
    from jax.sharding import Mesh, NamedSharding, PartitionSpec
    from concourse import bass2jax, mybir

    nc = _build()
    bass2jax.install_neuronx_cc_hook()

    partition_name = (
        nc.partition_id_tensor.name if nc.partition_id_tensor else None
    )
    in_names, out_names, out_avals = [], [], []
    for alloc in nc.m.functions[0].allocations:
        if not isinstance(alloc, mybir.MemoryLocationSet):
            continue
        if not alloc.memorylocations:
            continue
        name = alloc.memorylocations[0].name
        if alloc.kind == "ExternalInput":
            if name != partition_name:
                in_names.append(name)
        elif alloc.kind == "ExternalOutput":
            out_names.append(name)
            out_avals.append(
                jax.core.ShapedArray(
                    tuple(alloc.tensor_shape), mybir.dt.np(alloc.dtype)
                )
            )
    n_params = len(in_names)
    n_outs = len(out_names)
    # bass_exec operand order: inputs, then (donated) output buffers, then
    # partition id - mirrors run_bass_via_pjrt.
    all_names = list(in_names) + list(out_names)
    if partition_name is not None:
        all_names.append(partition_name)

    def _body(*args):
        operands = list(args)
        if partition_name is not None:
            operands.append(bass2jax.partition_id_tensor())
        outs = bass2jax._bass_exec_p.bind(
            *operands,
            out_avals=tuple(out_avals),
            in_names=tuple(all_names),
            out_names=tuple(out_names),
            lowering_input_output_aliases=(),
            sim_require_finite=True,
            sim_require_nnan=True,
            nc=nc,
        )
        return tuple(outs)

    devices = jax.devices()[:N_CORES]
    assert len(devices) == N_CORES, (
        f"need {N_CORES} devices, have {len(jax.devices())}"
    )
    mesh = Mesh(np.asarray(devices), ("core",))
    in_specs = (PartitionSpec("core"),) * (n_params + n_outs)
    out_specs = (PartitionSpec("core"),) * n_outs
    sharded = jax.jit(
        shard_map(
            _body, mesh=mesh, in_specs=in_specs, out_specs=out_specs,
            check_rep=False,
        ),
        donate_argnums=tuple(range(n_params, n_params + n_outs)),
        keep_unused=True,
    )
    # Donated output buffers materialize on-device (params-only bass_exec
    # module cannot contain a zeros op; a separate jit can).
    zsh = NamedSharding(mesh, PartitionSpec("core"))
    zavals = [
        (tuple([N_CORES * a.shape[0]] + list(a.shape[1:])), a.dtype)
        for a in out_avals
    ]
    zeros_fn = jax.jit(
        lambda: tuple(jnp.zeros(s, d) for s, d in zavals),
        out_shardings=tuple(zsh for _ in zavals),
    )
    _CACHE["exec"] = (sharded, zeros_fn, in_names, out_names)
    return _CACHE["exec"]


def kernel(**inputs):
    sharded, zeros_fn, in_names, out_names = _get_exec()

    bf16 = ml_dtypes.bfloat16
    x_enc = np.asarray(inputs["x_enc"], np.float32).reshape(B, T, E)
    x_prd = np.asarray(inputs["x_prd"], np.float32).reshape(B, U, E)
    shared = {
        "w_l": np.ascontiguousarray(
            np.asarray(inputs["w_l"], np.float32).astype(bf16)
        ),
        "b_l": np.ascontiguousarray(np.asarray(inputs["b_l"], np.float32)),
        "w_p": np.ascontiguousarray(
            np.asarray(inputs["w_p"], np.float32).astype(bf16)
        ),
        "b_p": np.ascontiguousarray(np.asarray(inputs["b_p"], np.float32)),
        "w_h": np.ascontiguousarray(
            np.asarray(inputs["w_h"], np.float32).astype(bf16)
        ),
        "b_h": np.ascontiguousarray(np.asarray(inputs["b_h"], np.float32)),
    }
    in_maps = []
    for b in range(N_CORES):
        m = dict(shared)
        m["xeT"] = np.ascontiguousarray(x_enc[b].T.astype(bf16))
        m["xpT"] = np.ascontiguousarray(x_prd[b].T.astype(bf16))
        in_maps.append(m)

    global _last_in_maps
    _last_in_maps = in_maps

    concat_in = [
        np.concatenate([in_maps[c][n] for c in range(N_CORES)], axis=0)
        for n in in_names
    ]
    out_arrs = sharded(*concat_in, *zeros_fn())
    out16 = np.asarray(out_arrs[out_names.index("out")])
    return out16.reshape(B, T, U, V).astype(np.float32)


# revision 14
# speedup vs baseline: 1.4979x; 1.0254x over previous
"""RNN-T JointNetwork kernel for Trainium2 (Bass/Tile), SPMD over 8 NeuronCores.

Computes, per batch element b (one per core):
    h_enc = x_enc[b] @ w_l + b_l          # (T, H)
    h_prd = x_prd[b] @ w_p + b_p          # (U, H)
    h     = tanh(h_enc[t] + h_prd[u])     # (T, U, H)
    out   = h @ w_h + b_h                 # (T, U, V)

Layout strategy (per core):
  * x_enc/x_prd arrive HOST-TRANSPOSED and in bf16: xeT [E, T], xpT [E, U]
    DMA straight into feature-major SBUF slabs - no PE transposes, no
    identity, no PSUM/ACT copies on the critical fill path.
  * Small GEMMs run in bf16 (1 cycle/row on the PE at any free size, vs 4
    for fp32r under 256); h_encT/h_prdT epilogues keep f32 for accuracy.
  * Rows of the big GEMM are ordered u-major: r' = u*T + t.  For a fixed u,
    h.T[:, u, :] = tanh(h_encT + h_prdT[:, u]) is ONE scalar-engine
    activation op (bias = per-partition column h_prdT[:, u]), fusing the
    broadcast-add and tanh; output hc is bf16 (PE-ready lhsT).
  * Big GEMM: bf16 stationary (hc) x bf16 moving (w_h), f32 PSUM, V split
    into two 512-wide PSUM banks, 4 k-tiles of H, 4 output tiles in flight
    (tags ps0/ps1 x bufs=4 = all 8 PSUM banks).
  * Epilogue: DVE adds b_h and narrows to fp16 into one [128, V] tile;
    output DMA moves HALF the bytes of an f32 kernel.  DMA transfers are a
    serial resource at ~360 GB/s aggregate, and the f32 output stream was
    the previous bottleneck (out DMA busy > PE busy).
  * Output rows r' = u*T + t map to logits rows r = t*U + u; each 128-row
    tile stores with <=2 DMAs (one per u-segment), 2 KB contiguous rows.

Host runner (wall-clock):
  * Caches the jitted shard_map executable across calls (no retrace).
  * Donated output buffers are created ON DEVICE by a tiny separate jit
    (the bass_exec HLO module must stay params-only), so the host never
    uploads zero-filled output-sized buffers.
  * Inputs ship as bf16 (x, w) + f32 biases; output returns as fp16 and is
    upcast host-side.  Per call: ~18 MB up + ~164 MB down instead of
    ~364 MB up + ~327 MB down.
"""

import sys

for _p in ("/opt/trn_rl_repo",):
    if _p not in sys.path:
        sys.path.insert(0, _p)

import numpy as np
import ml_dtypes

B, T, U = 8, 200, 50
E = H = 512
V = 1024
P = 128
KT = E // P  # 4 contraction tiles for the small GEMMs
HT = H // P  # 4 contraction tiles for the big GEMM
R = T * U    # rows per core
N_CORES = 8
CHUNKS = [16, 16, 16, 2]  # ragged 2-u chunk last: cheap 16-row drain tail

_CACHE = {}
_last_in_maps = None


def _emit(nc, tc, tile, mybir):
    f32 = mybir.dt.float32
    bf16 = mybir.dt.bfloat16
    f16 = mybir.dt.float16
    Act = mybir.ActivationFunctionType

    # Host-prepared layouts (see kernel() below) - one contiguous DMA per
    # SBUF slab (each DMA instruction pays ~0.6us HWDGE + ~0.9us semaphore
    # latency on the serial DMA path, so fewer/bigger is better):
    #   xeT  [P, KT*T]:      xeT[p, k*T+t] = x_enc[t, k*P+p]          (bf16)
    #   xpT  [P, KT*U]:      xpT[p, k*U+u] = x_prd[u, k*P+p]          (bf16)
    #   w_l  [P, HT*KT*P]:   w_l[p, (j*KT+k)*P+c] = w_l0[k*P+p, j*P+c] (bf16)
    #   w_p  same blocked permutation as w_l                           (bf16)
    #   b_lp [P, KT] f32:    b_lp[p, j] = b_l[j*P+p] + b_p[j*P+p]
    #   w_h  [P, HT*V]:      w_h[p, k*V+v] = w_h0[k*P+p, v]           (bf16)
    xeT_d = nc.dram_tensor("xeT", [P, KT * T], bf16, kind="ExternalInput")
    xpT_d = nc.dram_tensor("xpT", [P, KT * U], bf16, kind="ExternalInput")
    w_l_d = nc.dram_tensor("w_l", [P, HT * KT * P], bf16, kind="ExternalInput")
    w_p_d = nc.dram_tensor("w_p", [P, HT * KT * P], bf16, kind="ExternalInput")
    b_lp_d = nc.dram_tensor("b_lp", [P, KT], f32, kind="ExternalInput")
    w_h_d = nc.dram_tensor("w_h", [P, HT * V], bf16, kind="ExternalInput")
    b_h_d = nc.dram_tensor("b_h", [V], bf16, kind="ExternalInput")
    out_d = nc.dram_tensor("out", [R, V], f16, kind="ExternalOutput")

    from contextlib import ExitStack

    ctx = ExitStack()
    cpool = ctx.enter_context(tc.tile_pool(name="const", bufs=1))
    pbig = ctx.enter_context(tc.tile_pool(name="pbig", bufs=4, space="PSUM"))
    hcpool = ctx.enter_context(tc.tile_pool(name="hc", bufs=2))
    opool = ctx.enter_context(tc.tile_pool(name="op", bufs=6))

    # ---- input DMAs, ordered by first use (DMA engines are serial).
    # x_prd/w_p first: the h_prd -> tanh chain gates every big-GEMM tile.
    xpT = cpool.tile([P, KT * U], bf16, tag="xpT", name="xpT")
    nc.sync.dma_start(out=xpT[:], in_=xpT_d[:, :])
    wpA = cpool.tile([P, HT * KT * P], bf16, tag="wpA", name="wpA")
    nc.sync.dma_start(out=wpA[:], in_=w_p_d[:, :])
    wp = [wpA[:, j * KT * P:(j + 1) * KT * P] for j in range(HT)]
    blp = cpool.tile([P, KT], f32, tag="blp")
    nc.sync.dma_start(out=blp[:], in_=b_lp_d[:, :])
    xeT = cpool.tile([P, KT * T], bf16, tag="xeT", name="xeT")
    nc.sync.dma_start(out=xeT[:], in_=xeT_d[:, :])
    wlA = cpool.tile([P, HT * KT * P], bf16, tag="wlA", name="wlA")
    nc.sync.dma_start(out=wlA[:], in_=w_l_d[:, :])
    wl = [wlA[:, j * KT * P:(j + 1) * KT * P] for j in range(HT)]
    bh_rep = cpool.tile([P, V], bf16, tag="bh")
    nc.sync.dma_start(
        out=bh_rep[:], in_=b_h_d[:].unsqueeze(0).broadcast_to([P, V])
    )
    whA = cpool.tile([P, HT * V], bf16, tag="whA", name="whA")
    for h_ in range(2):
        nc.sync.dma_start(
            out=whA[:, h_ * 2 * V:(h_ + 1) * 2 * V],
            in_=w_h_d[:, h_ * 2 * V:(h_ + 1) * 2 * V],
        )
    wh = [whA[:, k * V:(k + 1) * V] for k in range(HT)]

    # warm-up: force the Tanh act-table load (1.3us) off the critical path,
    # as soon as the first DMA lands rather than at the first real ACT op
    actwarm = cpool.tile([P, 1], f32, tag="actwarm")
    nc.scalar.activation(actwarm[:], xpT[:, 0:1], Act.Tanh)

    # ---- small GEMMs: h_encT [H, T], h_prdT [H, U] (+bias via ACT) ----
    heT = [cpool.tile([P, T], f32, tag=f"heT{j}", name=f"heT{j}")
           for j in range(HT)]
    hpT = [cpool.tile([P, U], f32, tag=f"hpT{j}", name=f"hpT{j}")
           for j in range(HT)]
    for j in range(HT):
        ps = pbig.tile([P, 512], f32, tag=f"ps{j % 2}", name="pss")
        for k in range(KT):
            nc.tensor.matmul(
                ps[:, :U],
                wp[j][:, k * P:(k + 1) * P],
                xpT[:, k * U:(k + 1) * U],
                start=(k == 0),
                stop=(k == KT - 1),
            )
        nc.scalar.activation(
            hpT[j][:], ps[:, :U], Act.Identity, bias=blp[:, j:j + 1]
        )
    for j in range(HT):
        ps = pbig.tile([P, 512], f32, tag=f"ps{j % 2}", name="pss")
        for k in range(KT):
            nc.tensor.matmul(
                ps[:, :T],
                wl[j][:, k * P:(k + 1) * P],
                xeT[:, k * T:(k + 1) * T],
                start=(k == 0),
                stop=(k == KT - 1),
            )
        # b_l is folded into the h_prdT bias (tanh adds them anyway);
        # plain PSUM->SBUF copy on the otherwise-idle DVE
        nc.vector.tensor_copy(out=heT[j][:], in_=ps[:, :T])

    # ---- main loop over u-chunks; rows r' = u*T + t ----
    out_view = out_d[:].rearrange("(t u) v -> u t v", u=U)
    max_cu = max(CHUNKS)
    u0 = 0
    for cu in CHUNKS:
        rc = cu * T
        hc = [hcpool.tile([P, max_cu * T], bf16, tag=f"hc{j}", name=f"hc{j}")
              for j in range(HT)]
        # fused broadcast-add + tanh; du-outer so early GEMM tiles unblock
        for du in range(cu):
            for j in range(HT):
                nc.scalar.activation(
                    hc[j][:, du * T:(du + 1) * T],
                    heT[j][:, :T],
                    Act.Tanh,
                    bias=hpT[j][:, u0 + du:u0 + du + 1],
                )
        # big GEMM over 128-row tiles of this chunk
        for m0 in range(0, rc, P):
            m = min(P, rc - m0)
            ps0 = pbig.tile([P, 512], f32, tag="ps0")
            ps1 = pbig.tile([P, 512], f32, tag="ps1")
            for j in range(HT):
                lhsT = hc[j][:, m0:m0 + m]
                nc.tensor.matmul(
                    ps0[:m, :], lhsT, wh[j][:, 0:512],
                    start=(j == 0), stop=(j == HT - 1),
                )
                nc.tensor.matmul(
                    ps1[:m, :], lhsT, wh[j][:, 512:V],
                    start=(j == 0), stop=(j == HT - 1),
                )
            # epilogue: bias add + fp16 narrowing into one [P, V] tile,
            # then <=2 store DMAs (split at u boundaries)
            ot = opool.tile([P, V], f16, tag="ot", name="ot")
            nc.vector.tensor_add(
                ot[:m, 0:512], ps0[:m, :], bh_rep[:m, 0:512]
            )
            nc.vector.tensor_add(
                ot[:m, 512:V], ps1[:m, :], bh_rep[:m, 512:V]
            )
            seg = m0
            while seg < m0 + m:
                du = seg // T
                tA = seg % T
                seg_len = min(m0 + m, (du + 1) * T) - seg
                nc.sync.dma_start(
                    out=out_view[u0 + du, tA:tA + seg_len, :],
                    in_=ot[seg - m0:seg - m0 + seg_len, :],
                )
                seg += seg_len
        u0 += cu

    ctx.close()


def _build():
    if "nc" in _CACHE:
        return _CACHE["nc"]
    from concourse import bacc, mybir
    import concourse.tile as tile

    nc = bacc.Bacc("TRN2", target_bir_lowering=False, debug=False)
    with tile.TileContext(nc) as tc:
        _emit(nc, tc, tile, mybir)
    nc.compile()
    _CACHE["nc"] = nc
    return nc


def _get_exec():
    """Build (once) the cached jitted shard_map executable + device-zeros fn."""
    if "exec" in _CACHE:
        return _CACHE["exec"]
    import jax
    import jax.numpy as jnp
    from jax.experimental.shard_map import shard_map
    from jax.sharding import Mesh, NamedSharding, PartitionSpec
    from concourse import bass2jax, mybir

    nc = _build()
    bass2jax.install_neuronx_cc_hook()

    partition_name = (
        nc.partition_id_tensor.name if nc.partition_id_tensor else None
    )
    in_names, out_names, out_avals = [], [], []
    for alloc in nc.m.functions[0].allocations:
        if not isinstance(alloc, mybir.MemoryLocationSet):
            continue
        if not alloc.memorylocations:
            continue
        name = alloc.memorylocations[0].name
        if alloc.kind == "ExternalInput":
            if name != partition_name:
                in_names.append(name)
        elif alloc.kind == "ExternalOutput":
            out_names.append(name)
            out_avals.append(
                jax.core.ShapedArray(
                    tuple(alloc.tensor_shape), mybir.dt.np(alloc.dtype)
                )
            )
    n_params = len(in_names)
    n_outs = len(out_names)
    # bass_exec operand order: inputs, then (donated) output buffers, then
    # partition id - mirrors run_bass_via_pjrt.
    all_names = list(in_names) + list(out_names)
    if partition_name is not None:
        all_names.append(partition_name)

    def _body(*args):
        operands = list(args)
        if partition_name is not None:
            operands.append(bass2jax.partition_id_tensor())
        outs = bass2jax._bass_exec_p.bind(
            *operands,
            out_avals=tuple(out_avals),
            in_names=tuple(all_names),
            out_names=tuple(out_names),
            lowering_input_output_aliases=(),
            sim_require_finite=True,
            sim_require_nnan=True,
            nc=nc,
        )
        return tuple(outs)

    devices = jax.devices()[:N_CORES]
    assert len(devices) == N_CORES, (
        f"need {N_CORES} devices, have {len(jax.devices())}"
    )
    mesh = Mesh(np.asarray(devices), ("core",))
    in_specs = (PartitionSpec("core"),) * (n_params + n_outs)
    out_specs = (PartitionSpec("core"),) * n_outs
    sharded = jax.jit(
        shard_map(
            _body, mesh=mesh, in_specs=in_specs, out_specs=out_specs,
            check_rep=False,
        ),
        donate_argnums=tuple(range(n_params, n_params + n_outs)),
        keep_unused=True,
    )
    # Donated output buffers materialize on-device (params-only bass_exec
    # module cannot contain a zeros op; a separate jit can).
    zsh = NamedSharding(mesh, PartitionSpec("core"))
    zavals = [
        (tuple([N_CORES * a.shape[0]] + list(a.shape[1:])), a.dtype)
        for a in out_avals
    ]
    zeros_fn = jax.jit(
        lambda: tuple(jnp.zeros(s, d) for s, d in zavals),
        out_shardings=tuple(zsh for _ in zavals),
    )
    _CACHE["exec"] = (sharded, zeros_fn, in_names, out_names)
    return _CACHE["exec"]


def kernel(**inputs):
    sharded, zeros_fn, in_names, out_names = _get_exec()

    bf16 = ml_dtypes.bfloat16
    x_enc = np.asarray(inputs["x_enc"], np.float32).reshape(B, T, E)
    x_prd = np.asarray(inputs["x_prd"], np.float32).reshape(B, U, E)

    def _jmajor(w):
        # [E, H] -> [P, HT*KT*P] with w_out[p, (j*KT+k)*P+c] = w[k*P+p, j*P+c]
        w = np.asarray(w, np.float32).astype(bf16)
        return np.ascontiguousarray(
            w.reshape(KT, P, HT, P).transpose(1, 2, 0, 3)
            .reshape(P, HT * KT * P)
        )

    b_lp = (
        np.asarray(inputs["b_l"], np.float32)
        + np.asarray(inputs["b_p"], np.float32)
    ).reshape(KT, P).T
    shared = {
        "w_l": _jmajor(inputs["w_l"]),
        "w_p": _jmajor(inputs["w_p"]),
        "b_lp": np.ascontiguousarray(b_lp),
        # [H, V] -> [P, HT*V] with w_out[p, k*V+v] = w_h[k*P+p, v]
        "w_h": np.ascontiguousarray(
            np.asarray(inputs["w_h"], np.float32).astype(bf16)
            .reshape(HT, P, V).transpose(1, 0, 2).reshape(P, HT * V)
        ),
        "b_h": np.ascontiguousarray(
            np.asarray(inputs["b_h"], np.float32).astype(bf16)
        ),
    }
    in_maps = []
    for b in range(N_CORES):
        m = dict(shared)
        # [T, E] -> [P, KT*T] with x_out[p, k*T+t] = x[t, k*P+p]
        m["xeT"] = np.ascontiguousarray(
            x_enc[b].astype(bf16).reshape(T, KT, P)
            .transpose(2, 1, 0).reshape(P, KT * T)
        )
        m["xpT"] = np.ascontiguousarray(
            x_prd[b].astype(bf16).reshape(U, KT, P)
            .transpose(2, 1, 0).reshape(P, KT * U)
        )
        in_maps.append(m)

    global _last_in_maps
    _last_in_maps = in_maps

    concat_in = [
        np.concatenate([in_maps[c][n] for c in range(N_CORES)], axis=0)
        for n in in_names
    ]
    out_arrs = sharded(*concat_in, *zeros_fn())
    out_arr = out_arrs[out_names.index("out")]
    # Fetch per-core shards and upcast fp16->fp32 while the next shard is
    # still in flight (the PJRT transfer releases the GIL).
    try:
        import threading
        import queue

        shards = sorted(
            out_arr.addressable_shards, key=lambda s: s.index[0].start
        )
        assert len(shards) == N_CORES
        out = np.empty((B, T, U, V), np.float32)
        q = queue.Queue(2)

        def _fetch():
            for i, s in enumerate(shards):
                q.put((i, np.asarray(s.data)))
            q.put(None)

        th = threading.Thread(target=_fetch)
        th.start()
        while True:
            item = q.get()
            if item is None:
                break
            i, a = item
            out[i] = a.reshape(T, U, V)
        th.join()
        return out
    except Exception:
        out16 = np.asarray(out_arr)
        return out16.reshape(B, T, U, V).astype(np.float32)


# revision 16
# speedup vs baseline: 1.4998x; 1.0012x over previous
"""RNN-T JointNetwork kernel for Trainium2 (Bass/Tile), SPMD over 8 NeuronCores.

Computes, per batch element b (one per core):
    h_enc = x_enc[b] @ w_l + b_l          # (T, H)
    h_prd = x_prd[b] @ w_p + b_p          # (U, H)
    h     = tanh(h_enc[t] + h_prd[u])     # (T, U, H)
    out   = h @ w_h + b_h                 # (T, U, V)

Layout strategy (per core):
  * x_enc/x_prd arrive HOST-TRANSPOSED and in bf16: xeT [E, T], xpT [E, U]
    DMA straight into feature-major SBUF slabs - no PE transposes, no
    identity, no PSUM/ACT copies on the critical fill path.
  * Small GEMMs run in bf16 (1 cycle/row on the PE at any free size, vs 4
    for fp32r under 256); h_encT/h_prdT epilogues keep f32 for accuracy.
  * Rows of the big GEMM are ordered u-major: r' = u*T + t.  For a fixed u,
    h.T[:, u, :] = tanh(h_encT + h_prdT[:, u]) is ONE scalar-engine
    activation op (bias = per-partition column h_prdT[:, u]), fusing the
    broadcast-add and tanh; output hc is bf16 (PE-ready lhsT).
  * Big GEMM: bf16 stationary (hc) x bf16 moving (w_h), f32 PSUM, V split
    into two 512-wide PSUM banks, 4 k-tiles of H, 4 output tiles in flight
    (tags ps0/ps1 x bufs=4 = all 8 PSUM banks).
  * Epilogue: DVE adds b_h and narrows to fp16 into one [128, V] tile;
    output DMA moves HALF the bytes of an f32 kernel.  DMA transfers are a
    serial resource at ~360 GB/s aggregate, and the f32 output stream was
    the previous bottleneck (out DMA busy > PE busy).
  * Output rows r' = u*T + t map to logits rows r = t*U + u; each 128-row
    tile stores with <=2 DMAs (one per u-segment), 2 KB contiguous rows.

Host runner (wall-clock):
  * Caches the jitted shard_map executable across calls (no retrace).
  * Donated output buffers are created ON DEVICE by a tiny separate jit
    (the bass_exec HLO module must stay params-only), so the host never
    uploads zero-filled output-sized buffers.
  * Inputs ship as bf16 (x, w) + f32 biases; output returns as fp16 and is
    upcast host-side.  Per call: ~18 MB up + ~164 MB down instead of
    ~364 MB up + ~327 MB down.
"""

import sys

for _p in ("/opt/trn_rl_repo",):
    if _p not in sys.path:
        sys.path.insert(0, _p)

import numpy as np
import ml_dtypes

B, T, U = 8, 200, 50
E = H = 512
V = 1024
P = 128
KT = E // P  # 4 contraction tiles for the small GEMMs
HT = H // P  # 4 contraction tiles for the big GEMM
R = T * U    # rows per core
N_CORES = 8
CHUNKS = [16, 16, 16, 2]  # ragged 2-u chunk last: cheap 16-row drain tail

_CACHE = {}
_last_in_maps = None


def _emit(nc, tc, tile, mybir):
    f32 = mybir.dt.float32
    bf16 = mybir.dt.bfloat16
    f16 = mybir.dt.float16
    Act = mybir.ActivationFunctionType

    # Host-prepared layouts (see kernel() below) - one contiguous DMA per
    # SBUF slab (each DMA instruction pays ~0.6us HWDGE + ~0.9us semaphore
    # latency on the serial DMA path, so fewer/bigger is better):
    #   xeT  [P, KT*T]:      xeT[p, k*T+t] = x_enc[t, k*P+p]          (bf16)
    #   xpT  [P, KT*U]:      xpT[p, k*U+u] = x_prd[u, k*P+p]          (bf16)
    #   w_l  [P, HT*KT*P]:   w_l[p, (j*KT+k)*P+c] = w_l0[k*P+p, j*P+c] (bf16)
    #   w_p  same blocked permutation as w_l                           (bf16)
    #   b_lp [P, KT] f32:    b_lp[p, j] = b_l[j*P+p] + b_p[j*P+p]
    #   w_h  [P, HT*V]:      w_h[p, k*V+v] = w_h0[k*P+p, v]           (bf16)
    xeT_d = nc.dram_tensor("xeT", [P, KT * T], bf16, kind="ExternalInput")
    xpT_d = nc.dram_tensor("xpT", [P, KT * U], bf16, kind="ExternalInput")
    w_l_d = nc.dram_tensor("w_l", [P, HT * KT * P], bf16, kind="ExternalInput")
    w_p_d = nc.dram_tensor("w_p", [P, HT * KT * P], bf16, kind="ExternalInput")
    b_lp_d = nc.dram_tensor("b_lp", [P, KT], f32, kind="ExternalInput")
    w_h_d = nc.dram_tensor("w_h", [P, HT * V], bf16, kind="ExternalInput")
    b_h_d = nc.dram_tensor("b_h", [V], bf16, kind="ExternalInput")
    out_d = nc.dram_tensor("out", [R, V], f16, kind="ExternalOutput")

    from contextlib import ExitStack

    ctx = ExitStack()
    cpool = ctx.enter_context(tc.tile_pool(name="const", bufs=1))
    pbig = ctx.enter_context(tc.tile_pool(name="pbig", bufs=4, space="PSUM"))
    hcpool = ctx.enter_context(tc.tile_pool(name="hc", bufs=2))
    opool = ctx.enter_context(tc.tile_pool(name="op", bufs=6))

    # ---- input DMAs, ordered so both critical chains resolve together:
    # (x/w_l -> h_enc -> tanh) and (x_prd/w_p -> h_prd -> tanh) feed every
    # big-GEMM tile; w_h halves land just-in-time for tile0's j2/j3; b_h is
    # only needed by the first DVE epilogue, after that.
    xeT = cpool.tile([P, KT * T], bf16, tag="xeT", name="xeT")
    nc.sync.dma_start(out=xeT[:], in_=xeT_d[:, :])
    wlA = cpool.tile([P, HT * KT * P], bf16, tag="wlA", name="wlA")
    nc.sync.dma_start(out=wlA[:], in_=w_l_d[:, :])
    wl = [wlA[:, j * KT * P:(j + 1) * KT * P] for j in range(HT)]
    xpT = cpool.tile([P, KT * U], bf16, tag="xpT", name="xpT")
    nc.sync.dma_start(out=xpT[:], in_=xpT_d[:, :])
    wpA = cpool.tile([P, HT * KT * P], bf16, tag="wpA", name="wpA")
    nc.sync.dma_start(out=wpA[:], in_=w_p_d[:, :])
    wp = [wpA[:, j * KT * P:(j + 1) * KT * P] for j in range(HT)]
    blp = cpool.tile([P, KT], f32, tag="blp")
    nc.sync.dma_start(out=blp[:], in_=b_lp_d[:, :])
    whA = cpool.tile([P, HT * V], bf16, tag="whA", name="whA")
    for h_ in range(2):
        nc.sync.dma_start(
            out=whA[:, h_ * 2 * V:(h_ + 1) * 2 * V],
            in_=w_h_d[:, h_ * 2 * V:(h_ + 1) * 2 * V],
        )
    wh = [whA[:, k * V:(k + 1) * V] for k in range(HT)]
    bh_rep = cpool.tile([P, V], bf16, tag="bh")
    nc.sync.dma_start(
        out=bh_rep[:], in_=b_h_d[:].unsqueeze(0).broadcast_to([P, V])
    )

    # warm-up: force the Tanh act-table load (1.3us) off the critical path,
    # as soon as the first DMA lands rather than at the first real ACT op
    actwarm = cpool.tile([P, 1], f32, tag="actwarm")
    nc.scalar.activation(actwarm[:], xeT[:, 0:1], Act.Tanh)

    # ---- small GEMMs: h_encT [H, T], h_prdT [H, U] (+bias via ACT) ----
    heT = [cpool.tile([P, T], f32, tag=f"heT{j}", name=f"heT{j}")
           for j in range(HT)]
    hpT = [cpool.tile([P, U], f32, tag=f"hpT{j}", name=f"hpT{j}")
           for j in range(HT)]
    for j in range(HT):
        ps = pbig.tile([P, 512], f32, tag=f"ps{j % 2}", name="pss")
        for k in range(KT):
            nc.tensor.matmul(
                ps[:, :T],
                wl[j][:, k * P:(k + 1) * P],
                xeT[:, k * T:(k + 1) * T],
                start=(k == 0),
                stop=(k == KT - 1),
            )
        # b_l is folded into the h_prdT bias (tanh adds them anyway);
        # plain PSUM->SBUF copy on the otherwise-idle DVE
        nc.vector.tensor_copy(out=heT[j][:], in_=ps[:, :T])
    for j in range(HT):
        ps = pbig.tile([P, 512], f32, tag=f"ps{j % 2}", name="pss")
        for k in range(KT):
            nc.tensor.matmul(
                ps[:, :U],
                wp[j][:, k * P:(k + 1) * P],
                xpT[:, k * U:(k + 1) * U],
                start=(k == 0),
                stop=(k == KT - 1),
            )
        nc.scalar.activation(
            hpT[j][:], ps[:, :U], Act.Identity, bias=blp[:, j:j + 1]
        )

    # ---- main loop over u-chunks; rows r' = u*T + t ----
    out_view = out_d[:].rearrange("(t u) v -> u t v", u=U)
    max_cu = max(CHUNKS)
    u0 = 0
    for cu in CHUNKS:
        rc = cu * T
        hc = [hcpool.tile([P, max_cu * T], bf16, tag=f"hc{j}", name=f"hc{j}")
              for j in range(HT)]
        # fused broadcast-add + tanh; du-outer so early GEMM tiles unblock
        for du in range(cu):
            for j in range(HT):
                nc.scalar.activation(
                    hc[j][:, du * T:(du + 1) * T],
                    heT[j][:, :T],
                    Act.Tanh,
                    bias=hpT[j][:, u0 + du:u0 + du + 1],
                )
        # big GEMM over 128-row tiles of this chunk
        for m0 in range(0, rc, P):
            m = min(P, rc - m0)
            ps0 = pbig.tile([P, 512], f32, tag="ps0")
            ps1 = pbig.tile([P, 512], f32, tag="ps1")
            for j in range(HT):
                lhsT = hc[j][:, m0:m0 + m]
                nc.tensor.matmul(
                    ps0[:m, :], lhsT, wh[j][:, 0:512],
                    start=(j == 0), stop=(j == HT - 1),
                )
                nc.tensor.matmul(
                    ps1[:m, :], lhsT, wh[j][:, 512:V],
                    start=(j == 0), stop=(j == HT - 1),
                )
            # epilogue: bias add + fp16 narrowing into one [P, V] tile,
            # then <=2 store DMAs (split at u boundaries)
            ot = opool.tile([P, V], f16, tag="ot", name="ot")
            nc.vector.tensor_add(
                ot[:m, 0:512], ps0[:m, :], bh_rep[:m, 0:512]
            )
            nc.vector.tensor_add(
                ot[:m, 512:V], ps1[:m, :], bh_rep[:m, 512:V]
            )
            seg = m0
            while seg < m0 + m:
                du = seg // T
                tA = seg % T
                seg_len = min(m0 + m, (du + 1) * T) - seg
                nc.sync.dma_start(
                    out=out_view[u0 + du, tA:tA + seg_len, :],
                    in_=ot[seg - m0:seg - m0 + seg_len, :],
                )
                seg += seg_len
        u0 += cu

    ctx.close()


def _build():
    if "nc" in _CACHE:
        return _CACHE["nc"]
    from concourse import bacc, mybir
    import concourse.tile as tile

    nc = bacc.Bacc("TRN2", target_bir_lowering=False, debug=False)
    with tile.TileContext(nc) as tc:
        _emit(nc, tc, tile, mybir)
    nc.compile()
    _CACHE["nc"] = nc
    return nc


def _get_exec():
    """Build (once) the cached jitted shard_map executable + device-zeros fn."""
    if "exec" in _CACHE:
        return _CACHE["exec"]
    import jax
    import jax.numpy as jnp
    from jax.experimental.shard_map import shard_map
    from jax.sharding import Mesh, NamedSharding, PartitionSpec
    from concourse import bass2jax, mybir

    nc = _build()
    bass2jax.install_neuronx_cc_hook()

    partition_name = (
        nc.partition_id_tensor.name if nc.partition_id_tensor else None
    )
    in_names, out_names, out_avals = [], [], []
    for alloc in nc.m.functions[0].allocations:
        if not isinstance(alloc, mybir.MemoryLocationSet):
            continue
        if not alloc.memorylocations:
            continue
        name = alloc.memorylocations[0].name
        if alloc.kind == "ExternalInput":
            if name != partition_name:
                in_names.append(name)
        elif alloc.kind == "ExternalOutput":
            out_names.append(name)
            out_avals.append(
                jax.core.ShapedArray(
                    tuple(alloc.tensor_shape), mybir.dt.np(alloc.dtype)
                )
            )
    n_params = len(in_names)
    n_outs = len(out_names)
    # bass_exec operand order: inputs, then (donated) output buffers, then
    # partition id - mirrors run_bass_via_pjrt.
    all_names = list(in_names) + list(out_names)
    if partition_name is not None:
        all_names.append(partition_name)

    def _body(*args):
        operands = list(args)
        if partition_name is not None:
            operands.append(bass2jax.partition_id_tensor())
        outs = bass2jax._bass_exec_p.bind(
            *operands,
            out_avals=tuple(out_avals),
            in_names=tuple(all_names),
            out_names=tuple(out_names),
            lowering_input_output_aliases=(),
            sim_require_finite=True,
            sim_require_nnan=True,
            nc=nc,
        )
        return tuple(outs)

    devices = jax.devices()[:N_CORES]
    assert len(devices) == N_CORES, (
        f"need {N_CORES} devices, have {len(jax.devices())}"
    )
    mesh = Mesh(np.asarray(devices), ("core",))
    in_specs = (PartitionSpec("core"),) * (n_params + n_outs)
    out_specs = (PartitionSpec("core"),) * n_outs
    sharded = jax.jit(
        shard_map(
            _body, mesh=mesh, in_specs=in_specs, out_specs=out_specs,
            check_rep=False,
        ),
        donate_argnums=tuple(range(n_params, n_params + n_outs)),
        keep_unused=True,
    )
    # Donated output buffers materialize on-device (params-only bass_exec
    # module cannot contain a zeros op; a separate jit can).
    zsh = NamedSharding(mesh, PartitionSpec("core"))
    zavals = [
        (tuple([N_CORES * a.shape[0]] + list(a.shape[1:])), a.dtype)
        for a in out_avals
    ]
    zeros_fn = jax.jit(
        lambda: tuple(jnp.zeros(s, d) for s, d in zavals),
        out_shardings=tuple(zsh for _ in zavals),
    )
    _CACHE["exec"] = (sharded, zeros_fn, in_names, out_names)
    return _CACHE["exec"]


def kernel(**inputs):
    sharded, zeros_fn, in_names, out_names = _get_exec()

    bf16 = ml_dtypes.bfloat16
    x_enc = np.asarray(inputs["x_enc"], np.float32).reshape(B, T, E)
    x_prd = np.asarray(inputs["x_prd"], np.float32).reshape(B, U, E)

    def _jmajor(w):
        # [E, H] -> [P, HT*KT*P] with w_out[p, (j*KT+k)*P+c] = w[k*P+p, j*P+c]
        w = np.asarray(w, np.float32).astype(bf16)
        return np.ascontiguousarray(
            w.reshape(KT, P, HT, P).transpose(1, 2, 0, 3)
            .reshape(P, HT * KT * P)
        )

    b_lp = (
        np.asarray(inputs["b_l"], np.float32)
        + np.asarray(inputs["b_p"], np.float32)
    ).reshape(KT, P).T
    shared = {
        "w_l": _jmajor(inputs["w_l"]),
        "w_p": _jmajor(inputs["w_p"]),
        "b_lp": np.ascontiguousarray(b_lp),
        # [H, V] -> [P, HT*V] with w_out[p, k*V+v] = w_h[k*P+p, v]
        "w_h": np.ascontiguousarray(
            np.asarray(inputs["w_h"], np.float32).astype(bf16)
            .reshape(HT, P, V).transpose(1, 0, 2).reshape(P, HT * V)
        ),
        "b_h": np.ascontiguousarray(
            np.asarray(inputs["b_h"], np.float32).astype(bf16)
        ),
    }
    in_maps = []
    for b in range(N_CORES):
        m = dict(shared)
        # [T, E] -> [P, KT*T] with x_out[p, k*T+t] = x[t, k*P+p]
        m["xeT"] = np.ascontiguousarray(
            x_enc[b].astype(bf16).reshape(T, KT, P)
            .transpose(2, 1, 0).reshape(P, KT * T)
        )
        m["xpT"] = np.ascontiguousarray(
            x_prd[b].astype(bf16).reshape(U, KT, P)
            .transpose(2, 1, 0).reshape(P, KT * U)
        )
        in_maps.append(m)

    global _last_in_maps
    _last_in_maps = in_maps

    concat_in = [
        np.concatenate([in_maps[c][n] for c in range(N_CORES)], axis=0)
        for n in in_names
    ]
    out_arrs = sharded(*concat_in, *zeros_fn())
    out_arr = out_arrs[out_names.index("out")]
    # Fetch per-core shards and upcast fp16->fp32 while the next shard is
    # still in flight (the PJRT transfer releases the GIL).
    try:
        import threading
        import queue

        shards = sorted(
            out_arr.addressable_shards, key=lambda s: s.index[0].start
        )
        assert len(shards) == N_CORES
        out = np.empty((B, T, U, V), np.float32)
        q = queue.Queue(2)

        def _fetch():
            for i, s in enumerate(shards):
                q.put((i, np.asarray(s.data)))
            q.put(None)

        th = threading.Thread(target=_fetch)
        th.start()
        while True:
            item = q.get()
            if item is None:
                break
            i, a = item
            out[i] = a.reshape(T, U, V)
        th.join()
        return out
    except Exception:
        out16 = np.asarray(out_arr)
        return out16.reshape(B, T, U, V).astype(np.float32)


# revision 19
# speedup vs baseline: 1.5031x; 1.0022x over previous
"""RNN-T JointNetwork kernel for Trainium2 (Bass/Tile), SPMD over 8 NeuronCores.

Computes, per batch element b (one per core):
    h_enc = x_enc[b] @ w_l + b_l          # (T, H)
    h_prd = x_prd[b] @ w_p + b_p          # (U, H)
    h     = tanh(h_enc[t] + h_prd[u])     # (T, U, H)
    out   = h @ w_h + b_h                 # (T, U, V)

Layout strategy (per core):
  * x_enc/x_prd arrive HOST-TRANSPOSED and in bf16: xeT [E, T], xpT [E, U]
    DMA straight into feature-major SBUF slabs - no PE transposes, no
    identity, no PSUM/ACT copies on the critical fill path.
  * Small GEMMs run in bf16 (1 cycle/row on the PE at any free size, vs 4
    for fp32r under 256); h_encT/h_prdT epilogues keep f32 for accuracy.
  * Rows of the big GEMM are ordered u-major: r' = u*T + t.  For a fixed u,
    h.T[:, u, :] = tanh(h_encT + h_prdT[:, u]) is ONE scalar-engine
    activation op (bias = per-partition column h_prdT[:, u]), fusing the
    broadcast-add and tanh; output hc is bf16 (PE-ready lhsT).
  * Big GEMM: bf16 stationary (hc) x bf16 moving (w_h), f32 PSUM, V split
    into two 512-wide PSUM banks, 4 k-tiles of H, 4 output tiles in flight
    (tags ps0/ps1 x bufs=4 = all 8 PSUM banks).
  * Epilogue: DVE adds b_h and narrows to fp16 into one [128, V] tile;
    output DMA moves HALF the bytes of an f32 kernel.  DMA transfers are a
    serial resource at ~360 GB/s aggregate, and the f32 output stream was
    the previous bottleneck (out DMA busy > PE busy).
  * Output rows r' = u*T + t map to logits rows r = t*U + u; each 128-row
    tile stores with <=2 DMAs (one per u-segment), 2 KB contiguous rows.

Host runner (wall-clock):
  * Caches the jitted shard_map executable across calls (no retrace).
  * Donated output buffers are created ON DEVICE by a tiny separate jit
    (the bass_exec HLO module must stay params-only), so the host never
    uploads zero-filled output-sized buffers.
  * Inputs ship as bf16 (x, w) + f32 biases; output returns as fp16 and is
    upcast host-side.  Per call: ~18 MB up + ~164 MB down instead of
    ~364 MB up + ~327 MB down.
"""

import sys

for _p in ("/opt/trn_rl_repo",):
    if _p not in sys.path:
        sys.path.insert(0, _p)

import numpy as np
import ml_dtypes

B, T, U = 8, 200, 50
E = H = 512
V = 1024
P = 128
KT = E // P  # 4 contraction tiles for the small GEMMs
HT = H // P  # 4 contraction tiles for the big GEMM
R = T * U    # rows per core
N_CORES = 8
CHUNKS = [16, 16, 16, 2]  # ragged 2-u chunk last: cheap 16-row drain tail

_CACHE = {}
_last_in_maps = None


def _emit(nc, tc, tile, mybir):
    f32 = mybir.dt.float32
    bf16 = mybir.dt.bfloat16
    f16 = mybir.dt.float16
    Act = mybir.ActivationFunctionType

    # Host-prepared layouts (see kernel() below) - one contiguous DMA per
    # SBUF slab (each DMA instruction pays ~0.6us HWDGE + ~0.9us semaphore
    # latency on the serial DMA path, so fewer/bigger is better):
    #   xeT  [P, KT*T]:      xeT[p, k*T+t] = x_enc[t, k*P+p]          (bf16)
    #   xpT  [P, KT*U]:      xpT[p, k*U+u] = x_prd[u, k*P+p]          (bf16)
    #   w_l  [P, HT*KT*P]:   w_l[p, (j*KT+k)*P+c] = w_l0[k*P+p, j*P+c] (bf16)
    #   w_p  same blocked permutation as w_l                           (bf16)
    #   b_lp [P, KT] f32:    b_lp[p, j] = b_l[j*P+p] + b_p[j*P+p]
    #   w_h  [P, HT*V]:      w_h[p, k*V+v] = w_h0[k*P+p, v]           (bf16)
    xeT_d = nc.dram_tensor("xeT", [P, KT * T], bf16, kind="ExternalInput")
    xpT_d = nc.dram_tensor("xpT", [P, KT * U], bf16, kind="ExternalInput")
    w_l_d = nc.dram_tensor("w_l", [P, HT * KT * P], bf16, kind="ExternalInput")
    w_p_d = nc.dram_tensor("w_p", [P, HT * KT * P], bf16, kind="ExternalInput")
    b_lp_d = nc.dram_tensor("b_lp", [P, KT], f32, kind="ExternalInput")
    w_h_d = nc.dram_tensor("w_h", [P, HT * V], bf16, kind="ExternalInput")
    b_h_d = nc.dram_tensor("b_h", [V], bf16, kind="ExternalInput")
    out_d = nc.dram_tensor("out", [R, V], f16, kind="ExternalOutput")

    from contextlib import ExitStack

    ctx = ExitStack()
    cpool = ctx.enter_context(tc.tile_pool(name="const", bufs=1))
    pbig = ctx.enter_context(tc.tile_pool(name="pbig", bufs=4, space="PSUM"))
    hcpool = ctx.enter_context(tc.tile_pool(name="hc", bufs=2))
    opool = ctx.enter_context(tc.tile_pool(name="op", bufs=6))

    # ---- input DMAs, ordered so both critical chains resolve together:
    # (x/w_l -> h_enc -> tanh) and (x_prd/w_p -> h_prd -> tanh) feed every
    # big-GEMM tile; w_h halves land just-in-time for tile0's j2/j3; b_h is
    # only needed by the first DVE epilogue, after that.
    xeT = cpool.tile([P, KT * T], bf16, tag="xeT", name="xeT")
    nc.sync.dma_start(out=xeT[:], in_=xeT_d[:, :])
    wlA = cpool.tile([P, HT * KT * P], bf16, tag="wlA", name="wlA")
    nc.sync.dma_start(out=wlA[:], in_=w_l_d[:, :])
    wl = [wlA[:, j * KT * P:(j + 1) * KT * P] for j in range(HT)]
    xpT = cpool.tile([P, KT * U], bf16, tag="xpT", name="xpT")
    nc.sync.dma_start(out=xpT[:], in_=xpT_d[:, :])
    wpA = cpool.tile([P, HT * KT * P], bf16, tag="wpA", name="wpA")
    nc.sync.dma_start(out=wpA[:], in_=w_p_d[:, :])
    wp = [wpA[:, j * KT * P:(j + 1) * KT * P] for j in range(HT)]
    blp = cpool.tile([P, KT], f32, tag="blp")
    nc.sync.dma_start(out=blp[:], in_=b_lp_d[:, :])
    whA = cpool.tile([P, HT * V], bf16, tag="whA", name="whA")
    for h_ in range(2):
        nc.sync.dma_start(
            out=whA[:, h_ * 2 * V:(h_ + 1) * 2 * V],
            in_=w_h_d[:, h_ * 2 * V:(h_ + 1) * 2 * V],
        )
    wh = [whA[:, k * V:(k + 1) * V] for k in range(HT)]
    bh_rep = cpool.tile([P, V], bf16, tag="bh")
    nc.sync.dma_start(
        out=bh_rep[:], in_=b_h_d[:].unsqueeze(0).broadcast_to([P, V])
    )

    # warm-up: force the Tanh act-table load (1.3us) off the critical path,
    # as soon as the first DMA lands rather than at the first real ACT op
    actwarm = cpool.tile([P, 1], f32, tag="actwarm")
    nc.scalar.activation(actwarm[:], xeT[:, 0:1], Act.Tanh)

    # ---- small GEMMs: h_encT [H, T], h_prdT [H, U] (+bias via ACT) ----
    heT = [cpool.tile([P, T], f32, tag=f"heT{j}", name=f"heT{j}")
           for j in range(HT)]
    hpT = [cpool.tile([P, U], f32, tag=f"hpT{j}", name=f"hpT{j}")
           for j in range(HT)]
    for j in range(HT):
        ps = pbig.tile([P, 512], f32, tag=f"ps{j % 2}", name="pss")
        for k in range(KT):
            nc.tensor.matmul(
                ps[:, :T],
                wl[j][:, k * P:(k + 1) * P],
                xeT[:, k * T:(k + 1) * T],
                start=(k == 0),
                stop=(k == KT - 1),
            )
        # b_l is folded into the h_prdT bias (tanh adds them anyway);
        # plain PSUM->SBUF copy on the otherwise-idle DVE
        nc.vector.tensor_copy(out=heT[j][:], in_=ps[:, :T])
    for j in range(HT):
        ps = pbig.tile([P, 512], f32, tag=f"ps{j % 2}", name="pss")
        for k in range(KT):
            nc.tensor.matmul(
                ps[:, :U],
                wp[j][:, k * P:(k + 1) * P],
                xpT[:, k * U:(k + 1) * U],
                start=(k == 0),
                stop=(k == KT - 1),
            )
        nc.scalar.activation(
            hpT[j][:], ps[:, :U], Act.Identity, bias=blp[:, j:j + 1]
        )

    # ---- main loop over u-chunks; rows r' = u*T + t ----
    out_view = out_d[:].rearrange("(t u) v -> u t v", u=U)
    max_cu = max(CHUNKS)
    u0 = 0
    for cu in CHUNKS:
        rc = cu * T
        hc = [hcpool.tile([P, max_cu * T], bf16, tag=f"hc{j}", name=f"hc{j}")
              for j in range(HT)]
        # fused broadcast-add + tanh; du-outer so early GEMM tiles unblock
        for du in range(cu):
            for j in range(HT):
                nc.scalar.activation(
                    hc[j][:, du * T:(du + 1) * T],
                    heT[j][:, :T],
                    Act.Tanh,
                    bias=hpT[j][:, u0 + du:u0 + du + 1],
                )
        # big GEMM over 128-row tiles of this chunk
        for m0 in range(0, rc, P):
            m = min(P, rc - m0)
            ps0 = pbig.tile([P, 512], f32, tag="ps0")
            ps1 = pbig.tile([P, 512], f32, tag="ps1")
            # sequential per-bank passes: ps0 stops 4 matmuls early, so its
            # DVE epilogue overlaps the ps1 pass (shortens the drain tail)
            for j in range(HT):
                nc.tensor.matmul(
                    ps0[:m, :], hc[j][:, m0:m0 + m], wh[j][:, 0:512],
                    start=(j == 0), stop=(j == HT - 1),
                )
            for j in range(HT):
                nc.tensor.matmul(
                    ps1[:m, :], hc[j][:, m0:m0 + m], wh[j][:, 512:V],
                    start=(j == 0), stop=(j == HT - 1),
                )
            # epilogue: bias add + fp16 narrowing into one [P, V] tile,
            # then <=2 store DMAs (split at u boundaries)
            ot = opool.tile([P, V], f16, tag="ot", name="ot")
            nc.vector.tensor_add(
                ot[:m, 0:512], ps0[:m, :], bh_rep[:m, 0:512]
            )
            nc.vector.tensor_add(
                ot[:m, 512:V], ps1[:m, :], bh_rep[:m, 512:V]
            )
            seg = m0
            while seg < m0 + m:
                du = seg // T
                tA = seg % T
                seg_len = min(m0 + m, (du + 1) * T) - seg
                nc.sync.dma_start(
                    out=out_view[u0 + du, tA:tA + seg_len, :],
                    in_=ot[seg - m0:seg - m0 + seg_len, :],
                )
                seg += seg_len
        u0 += cu

    ctx.close()


def _build():
    if "nc" in _CACHE:
        return _CACHE["nc"]
    from concourse import bacc, mybir
    import concourse.tile as tile

    nc = bacc.Bacc("TRN2", target_bir_lowering=False, debug=False)
    with tile.TileContext(nc) as tc:
        _emit(nc, tc, tile, mybir)
    nc.compile()
    _CACHE["nc"] = nc
    return nc


def _get_exec():
    """Build (once) the cached jitted shard_map executable + device-zeros fn."""
    if "exec" in _CACHE:
        return _CACHE["exec"]
    import jax
    import jax.numpy as jnp
    from jax.experimental.shard_map import shard_map
    from jax.sharding import Mesh, NamedSharding, PartitionSpec
    from concourse import bass2jax, mybir

    nc = _build()
    bass2jax.install_neuronx_cc_hook()

    partition_name = (
        nc.partition_id_tensor.name if nc.partition_id_tensor else None
    )
    in_names, out_names, out_avals = [], [], []
    for alloc in nc.m.functions[0].allocations:
        if not isinstance(alloc, mybir.MemoryLocationSet):
            continue
        if not alloc.memorylocations:
            continue
        name = alloc.memorylocations[0].name
        if alloc.kind == "ExternalInput":
            if name != partition_name:
                in_names.append(name)
        elif alloc.kind == "ExternalOutput":
            out_names.append(name)
            out_avals.append(
                jax.core.ShapedArray(
                    tuple(alloc.tensor_shape), mybir.dt.np(alloc.dtype)
                )
            )
    n_params = len(in_names)
    n_outs = len(out_names)
    # bass_exec operand order: inputs, then (donated) output buffers, then
    # partition id - mirrors run_bass_via_pjrt.
    all_names = list(in_names) + list(out_names)
    if partition_name is not None:
        all_names.append(partition_name)

    def _body(*args):
        operands = list(args)
        if partition_name is not None:
            operands.append(bass2jax.partition_id_tensor())
        outs = bass2jax._bass_exec_p.bind(
            *operands,
            out_avals=tuple(out_avals),
            in_names=tuple(all_names),
            out_names=tuple(out_names),
            lowering_input_output_aliases=(),
            sim_require_finite=True,
            sim_require_nnan=True,
            nc=nc,
        )
        return tuple(outs)

    devices = jax.devices()[:N_CORES]
    assert len(devices) == N_CORES, (
        f"need {N_CORES} devices, have {len(jax.devices())}"
    )
    mesh = Mesh(np.asarray(devices), ("core",))
    in_specs = (PartitionSpec("core"),) * (n_params + n_outs)
    out_specs = (PartitionSpec("core"),) * n_outs
    sharded = jax.jit(
        shard_map(
            _body, mesh=mesh, in_specs=in_specs, out_specs=out_specs,
            check_rep=False,
        ),
        donate_argnums=tuple(range(n_params, n_params + n_outs)),
        keep_unused=True,
    )
    # Donated output buffers materialize on-device (params-only bass_exec
    # module cannot contain a zeros op; a separate jit can).
    zsh = NamedSharding(mesh, PartitionSpec("core"))
    zavals = [
        (tuple([N_CORES * a.shape[0]] + list(a.shape[1:])), a.dtype)
        for a in out_avals
    ]
    zeros_fn = jax.jit(
        lambda: tuple(jnp.zeros(s, d) for s, d in zavals),
        out_shardings=tuple(zsh for _ in zavals),
    )
    _CACHE["exec"] = (sharded, zeros_fn, in_names, out_names)
    return _CACHE["exec"]


def kernel(**inputs):
    sharded, zeros_fn, in_names, out_names = _get_exec()

    bf16 = ml_dtypes.bfloat16
    x_enc = np.asarray(inputs["x_enc"], np.float32).reshape(B, T, E)
    x_prd = np.asarray(inputs["x_prd"], np.float32).reshape(B, U, E)

    def _jmajor(w):
        # [E, H] -> [P, HT*KT*P] with w_out[p, (j*KT+k)*P+c] = w[k*P+p, j*P+c]
        w = np.asarray(w, np.float32).astype(bf16)
        return np.ascontiguousarray(
            w.reshape(KT, P, HT, P).transpose(1, 2, 0, 3)
            .reshape(P, HT * KT * P)
        )

    b_lp = (
        np.asarray(inputs["b_l"], np.float32)
        + np.asarray(inputs["b_p"], np.float32)
    ).reshape(KT, P).T
    shared = {
        "w_l": _jmajor(inputs["w_l"]),
        "w_p": _jmajor(inputs["w_p"]),
        "b_lp": np.ascontiguousarray(b_lp),
        # [H, V] -> [P, HT*V] with w_out[p, k*V+v] = w_h[k*P+p, v]
        "w_h": np.ascontiguousarray(
            np.asarray(inputs["w_h"], np.float32).astype(bf16)
            .reshape(HT, P, V).transpose(1, 0, 2).reshape(P, HT * V)
        ),
        "b_h": np.ascontiguousarray(
            np.asarray(inputs["b_h"], np.float32).astype(bf16)
        ),
    }
    in_maps = []
    for b in range(N_CORES):
        m = dict(shared)
        # [T, E] -> [P, KT*T] with x_out[p, k*T+t] = x[t, k*P+p]
        m["xeT"] = np.ascontiguousarray(
            x_enc[b].astype(bf16).reshape(T, KT, P)
            .transpose(2, 1, 0).reshape(P, KT * T)
        )
        m["xpT"] = np.ascontiguousarray(
            x_prd[b].astype(bf16).reshape(U, KT, P)
            .transpose(2, 1, 0).reshape(P, KT * U)
        )
        in_maps.append(m)

    global _last_in_maps
    _last_in_maps = in_maps

    concat_in = [
        np.concatenate([in_maps[c][n] for c in range(N_CORES)], axis=0)
        for n in in_names
    ]
    out_arrs = sharded(*concat_in, *zeros_fn())
    out_arr = out_arrs[out_names.index("out")]
    # Fetch per-core shards and upcast fp16->fp32 while the next shard is
    # still in flight (the PJRT transfer releases the GIL).
    try:
        import threading
        import queue

        shards = sorted(
            out_arr.addressable_shards, key=lambda s: s.index[0].start
        )
        assert len(shards) == N_CORES
        out = np.empty((B, T, U, V), np.float32)
        q = queue.Queue(2)

        def _fetch():
            for i, s in enumerate(shards):
                q.put((i, np.asarray(s.data)))
            q.put(None)

        th = threading.Thread(target=_fetch)
        th.start()
        while True:
            item = q.get()
            if item is None:
                break
            i, a = item
            out[i] = a.reshape(T, U, V)
        th.join()
        return out
    except Exception:
        out16 = np.asarray(out_arr)
        return out16.reshape(B, T, U, V).astype(np.float32)
